# revision 18
# baseline (speedup 1.0000x reference)
"""MoCo hard-example-mining loss (topk_masking) on 8 Trainium2 NeuronCores.

Strategy (sharding_hint: shard queue along K):
  After the enqueue step the effective queue is [feat_k.T | Z] where
  Z = queue[:, 512:] are L2-normalized columns with label 0.  The loss only
  needs, per feat_q row i, the max and min of p_ij = <feat_q_i, z_j> over
  the zero-label region (64.5K columns), plus exact handling of the 512
  special columns (done on the host in float64).

  Rows are PERMUTED on the host so that all rows with target==0 come first
  (m-block 0 of the matmul).  Only those rows need the region MIN (for them
  the region columns are positives); every row needs the region MAX.

  Device (per core, 1/8 of the columns = 8192, padded w/ duplicates):
    fp8e4 DoubleRow matmuls: psum = (16*q)·(16*z) = 256*p, 32 units of
    [128 rows, 2 banks=1024 cols].  Per unit one of:
      M (m-block 0): ACT exp(T/256*psum)+accum -> LSE max est;
                     DVE tensor_reduce min on psum -> exact min
      A: ACT exp+accum only (LSE max estimate)
      P: DVE tensor_reduce max on psum (exact max)
  Host combines: max-estimate = max over units/cores of log(sum)/T or
  raw TR max (/256); min likewise from M units.  Measured loss error is
  ~1e-5 relative (gate is 2e-2); LSE bias ~0.12 in p units largely
  cancels through the sqrt/softplus chain.
  Measured: ~47-48.3us HW exec (traced) vs 93.7us for the previous
  kernel under identical measurement (79.9us untraced harness number).
"""

import sys
import types
import numpy as np
import ml_dtypes

N, DIM, K, B = 512, 512, 65536, 512
NCORES = 8
KZ = K - B              # zero-label columns
CPC = K // NCORES       # padded columns per core (8192)
NT = CPC // 512         # 512-wide column tiles per core
BIG = 9999999.0

SQ = 16.0               # fp8 scale for feat_q
SZ = 16.0               # fp8 scale for queue columns
T = 8.0                 # LSE temperature (p units)
ACT_SCALE = T / (SQ * SZ)

FP8 = ml_dtypes.float8_e4m3

# types for the 6 non-m0 units per block, unit order
# [(1,0),(2,0),(3,0),(1,1),(2,1),(3,1)]
BLOCK_TYPES = {
    0: list("APPAPP"),
    1: list("APAPAP"),
    2: list("APAPAP"),
    3: list("APAPAP"),
}

LAST_RESULTS = None     # BassKernelResults of the most recent device run
_NC_CACHE = {}


def _install_axon_hooks_shim():
    """antenv.axon_hooks is absent on this image; bass_utils imports it when
    NTFF tracing is requested.  Provide the get/set module and register the
    ctypes-based NTFF hook so trace=True / BASS_TRACE=1 works."""
    try:
        import antenv  # noqa: F401
    except ImportError:
        return
    if "antenv.axon_hooks" in sys.modules:
        return
    mod = types.ModuleType("antenv.axon_hooks")
    mod._hook = None

    def set_axon_ntff_profile_hook(h):
        mod._hook = h

    def get_axon_ntff_profile_hook():
        return mod._hook

    mod.set_axon_ntff_profile_hook = set_axon_ntff_profile_hook
    mod.get_axon_ntff_profile_hook = get_axon_ntff_profile_hook
    sys.modules["antenv.axon_hooks"] = mod
    sys.modules["antenv"].axon_hooks = mod
    try:
        from trn_agent_boot.trn_boot import _ntff_profile_via_ctypes

        mod._hook = _ntff_profile_via_ctypes("/opt/axon/libaxon_pjrt.so")
    except Exception:
        pass


def _unit_list(nb):
    """units in production order: (m, h, type)"""
    out = []
    for h in range(2):
        for m in range(4):
            if m == 0:
                ty = "M"
            else:
                ty = BLOCK_TYPES[nb][(m - 1) + 3 * h]
            out.append((m, h, ty))
    return out


def _build_nc():
    import concourse.bacc as bacc
    import concourse.mybir as mybir
    from concourse.tile import TileContext

    fp8 = mybir.dt.float8e4
    bf16 = mybir.dt.bfloat16
    f32 = mybir.dt.float32

    nc = bacc.Bacc("TRN2", debug=False, target_bir_lowering=False)
    qt_d = nc.dram_tensor("qt", [128, 4 * 512], fp8, kind="ExternalInput")
    st_d = nc.dram_tensor("st", [128, NT * 4 * 512], fp8, kind="ExternalInput")
    out_d = nc.dram_tensor("out", [128, 96], f32, kind="ExternalOutput")

    qt_v = qt_d.ap().rearrange("p (k m) -> p k m", k=4)
    st_v = st_d.ap().rearrange("p (j t k c) -> p j t k c", j=4, t=4, k=4)

    with TileContext(nc) as tc:
        with (
            tc.tile_pool(name="qpool", bufs=1) as qpool,
            tc.tile_pool(name="spool", bufs=8) as spool,
            tc.tile_pool(name="bpool", bufs=2) as bpool,
            tc.tile_pool(name="opool", bufs=1) as opool,
            tc.tile_pool(name="pspool", bufs=4, space="PSUM") as pspool,
        ):
            qt = qpool.tile([128, 4, 512], fp8, name="qt")
            stq = [
                spool.tile([128, 4, 4, 512], fp8, name="stq", tag="stq", bufs=4)
                for _ in range(4)
            ]
            nc.sync.dma_start(out=qt, in_=qt_v)
            # first quad split in halves so block 0 can start early
            nc.sync.dma_start(out=stq[0][:, 0:2, :, :], in_=st_v[:, 0, 0:2, :, :])
            nc.sync.dma_start(out=stq[0][:, 2:4, :, :], in_=st_v[:, 0, 2:4, :, :])
            for j in range(1, 4):
                nc.sync.dma_start(out=stq[j], in_=st_v[:, j, :, :, :])

            # HAM warmup: wide cold matmuls filling the DMA-wait window
            warm = qpool.tile([128, 16], bf16, name="warm")
            warm2 = qpool.tile([128, 512], bf16, name="warm2")
            nc.vector.memset(warm, 0.0)
            nc.vector.memset(warm2, 0.0)
            wps = pspool.tile([128, 2, 512], f32, name="wps", tag="ps")
            for _ in range(8):
                nc.tensor.matmul(wps[0:16, 0, :], warm, warm2)
            # preload the exp activation table during the DMA wait
            wexpo = qpool.tile([128, 8], bf16, name="wexpo")
            nc.scalar.activation(wexpo, warm[:, 0:8],
                                 mybir.ActivationFunctionType.Exp)

            outt = opool.tile([128, 96], f32, name="outt")
            nc.gpsimd.memset(outt, 0.0)
            osum = outt[:, 0:32]
            ored = outt[:, 32:96]

            for nb in range(4):
                for (m, h, ty) in _unit_list(nb):
                    u = nb * 8 + m * 2 + h
                    ps = pspool.tile([128, 2, 512], f32, name="ps", tag="ps")
                    for kp in range(2):
                        for nn in range(2):
                            nc.tensor.matmul(
                                ps[:, nn, :],
                                qt[:, 2 * kp : 2 * kp + 2, m * 128 : (m + 1) * 128],
                                stq[nb][:, 2 * h + nn, 2 * kp : 2 * kp + 2, :],
                                start=(kp == 0),
                                stop=(kp == 1),
                                perf_mode=mybir.MatmulPerfMode.DoubleRow,
                            )
                    if ty in ("M", "A"):
                        bt = bpool.tile([128, 2, 512], bf16, name="bt", tag="bt")
                        nc.scalar.activation(
                            bt, ps, mybir.ActivationFunctionType.Exp,
                            scale=ACT_SCALE, accum_out=osum[:, u : u + 1],
                        )
                    if ty == "M":
                        # min of exp tile (SBUF) = exp(min); psum released by ACT
                        nc.vector.tensor_reduce(
                            ored[:, 2 * u : 2 * u + 2], bt,
                            axis=mybir.AxisListType.X, op=mybir.AluOpType.min,
                        )
                    elif ty == "P":
                        nc.vector.tensor_reduce(
                            ored[:, 2 * u : 2 * u + 2], ps,
                            axis=mybir.AxisListType.X, op=mybir.AluOpType.max,
                        )

            nc.scalar.dma_start(out=out_d.ap(), in_=outt)

    nc.compile()
    return nc


def _get_nc():
    if "nc" not in _NC_CACHE:
        _install_axon_hooks_shim()
        _NC_CACHE["nc"] = _build_nc()
    return _NC_CACHE["nc"]


def _loss(dist_ap, dist_an):
    diff = dist_an - dist_ap
    loss_soft = np.mean(np.logaddexp(0.0, -diff))
    if np.isinf(loss_soft):
        return np.float32(np.mean(np.maximum(dist_ap - dist_an + 0.3, 0.0)))
    return np.float32(loss_soft)


def _host_reference(feat_q, feat_k, targets, queue, queue_label):
    """Exact numpy fallback (float64) — used if structural assumptions or
    numeric sanity checks fail."""
    fq = feat_q.astype(np.float64)
    fk = feat_k.astype(np.float64)
    t = targets.astype(np.int64)
    q = queue.astype(np.float64).copy()
    ql = queue_label.astype(np.int64).copy()
    q[:, : fk.shape[0]] = fk.T
    ql[: fk.shape[0]] = t
    xx = (fq * fq).sum(1)[:, None]
    yy = (q * q).sum(0)[None, :]
    sq = xx + yy - 2.0 * (fq @ q)
    dist = np.sqrt(np.clip(sq, 1e-12, None))
    is_pos = t[:, None] == ql[None, :]
    dist_ap = np.max(dist - BIG * (~is_pos), axis=1)
    dist_an = np.min(dist + BIG * is_pos, axis=1)
    return _loss(dist_ap, dist_an)


def kernel(feat_q, feat_k, targets, queue, queue_label):
    feat_q = np.asarray(feat_q, dtype=np.float32)
    feat_k = np.asarray(feat_k, dtype=np.float32)
    targets = np.asarray(targets)
    queue = np.asarray(queue, dtype=np.float32)
    queue_label = np.asarray(queue_label)

    t = targets.astype(np.int64)
    Z = queue[:, B:]  # zero-label region, untouched by the enqueue

    # Guards for the structural assumptions this split relies on.
    ok = not np.any(queue_label != 0)
    if ok:
        sample = np.linspace(0, KZ - 1, 512, dtype=np.int64)
        yy_s = np.einsum("ij,ij->j", Z[:, sample], Z[:, sample], dtype=np.float64)
        ok = bool(np.max(np.abs(yy_s - 1.0)) < 1e-3)
    # Row permutation: target==0 rows first (they are the only ones needing
    # the region min, computed on m-block 0 of the device matmul).
    t0_rows = np.where(t == 0)[0]
    rest = np.where(t != 0)[0]
    if ok:
        ok = len(t0_rows) <= 128
    if not ok:
        return _host_reference(feat_q, feat_k, targets, queue, queue_label)

    perm = np.concatenate([t0_rows, rest])
    n0 = len(t0_rows)
    fq_p = feat_q[perm]
    t_p = t[perm]

    # ---- device inputs ----
    q8 = (SQ * fq_p.T).astype(FP8)          # [512 d, 512 i]
    qt_host = np.ascontiguousarray(
        q8.reshape(4, 128, 512).transpose(1, 0, 2).reshape(128, 4 * 512)
    )
    Z8 = (SZ * Z).astype(FP8)               # [512, 65024]
    in_maps = []
    for c in range(NCORES):
        lo = c * CPC
        hi = min((c + 1) * CPC, KZ)
        sl = np.empty((DIM, CPC), dtype=FP8)
        sl[:, : hi - lo] = Z8[:, lo:hi]
        if hi - lo < CPC:  # pad the tail core with duplicate columns
            sl[:, hi - lo :] = Z8[:, : CPC - (hi - lo)]
        st_host = np.ascontiguousarray(
            sl.reshape(4, 128, NT, 512).transpose(1, 2, 0, 3)
            .reshape(128, NT * 4 * 512)
        )
        in_maps.append({"qt": qt_host, "st": st_host})

    from concourse import bass_utils

    nc = _get_nc()
    res = bass_utils.run_bass_kernel_spmd(nc, in_maps, core_ids=list(range(NCORES)))
    global LAST_RESULTS
    LAST_RESULTS = res

    # ---- decode device outputs: raw units are 256*p ----
    pmax_raw = np.full((128, 4), -np.inf)
    pmin0_raw = np.full(128, np.inf)
    sums_ok = True
    for c in range(NCORES):
        outt = np.asarray(res.results[c]["out"], dtype=np.float64)
        osum = outt[:, 0:32]
        ored = outt[:, 32:96]
        for nb in range(4):
            for (m, h, ty) in _unit_list(nb):
                u = nb * 8 + m * 2 + h
                if ty in ("M", "A"):
                    s = osum[:, u]
                    if not np.all(np.isfinite(s)) or np.any(s <= 0.0):
                        sums_ok = False
                    else:
                        pmax_raw[:, m] = np.maximum(
                            pmax_raw[:, m], np.log(s) / T * (SQ * SZ)
                        )
                else:
                    pmax_raw[:, m] = np.maximum(
                        pmax_raw[:, m], ored[:, 2 * u : 2 * u + 2].max(axis=1)
                    )
                if ty == "M":
                    me = ored[:, 2 * u : 2 * u + 2].min(axis=1)
                    if np.any(me <= 0.0) or not np.all(np.isfinite(me)):
                        sums_ok = False
                    else:
                        pmin0_raw = np.minimum(
                            pmin0_raw, np.log(me) / T * (SQ * SZ)
                        )
    if not sums_ok or not np.all(np.isfinite(pmax_raw)):
        return _host_reference(feat_q, feat_k, targets, queue, queue_label)

    pmax = pmax_raw.T.reshape(N) / (SQ * SZ)     # row i_p = m*128 + p
    pmin0 = pmin0_raw / (SQ * SZ)                # rows 0..127 (m-block 0)

    # ---- host part: special 512-column block, exact in float64 ----
    fq = fq_p.astype(np.float64)
    fk = feat_k.astype(np.float64)
    xx = (fq * fq).sum(1)
    kk_ = (fk * fk).sum(1)
    G = fq @ fk.T
    sqB = xx[:, None] + kk_[None, :] - 2.0 * G
    distB = np.sqrt(np.clip(sqB, 1e-12, None))
    maskB = t_p[:, None] == t[None, :]           # special-block labels = targets
    apB = np.max(distB - BIG * (~maskB), axis=1)
    anB = np.min(distB + BIG * maskB, axis=1)

    # region contributions (columns are all label 0, unit norm)
    # rows with t_p != 0: region is negative -> min distance from pmax
    d_zmin = np.sqrt(np.clip(xx + 1.0 - 2.0 * pmax, 1e-12, None))
    an_z = d_zmin + BIG * (t_p == 0)
    # rows with t_p == 0 (first n0 rows): region is positive -> max distance
    ap_z = np.full(N, -BIG)
    if n0 > 0:
        d_zmax0 = np.sqrt(np.clip(xx[:n0] + 1.0 - 2.0 * pmin0[:n0], 1e-12, None))
        ap_z[:n0] = d_zmax0

    dist_ap = np.maximum(apB, ap_z)
    dist_an = np.minimum(anB, an_z)
    return _loss(dist_ap, dist_an)


# revision 19
# speedup vs baseline: 1.0586x; 1.0586x over previous
"""MoCo hard-example-mining loss (topk_masking) on 8 Trainium2 NeuronCores.

Strategy (sharding_hint: shard queue along K):
  After the enqueue step the effective queue is [feat_k.T | Z] where
  Z = queue[:, 512:] are L2-normalized columns with label 0.  The loss only
  needs, per feat_q row i, the max and min of p_ij = <feat_q_i, z_j> over
  the zero-label region (64.5K columns), plus exact handling of the 512
  special columns (done on the host in float64).

  Rows are PERMUTED on the host so that all rows with target==0 come first
  (m-block 0 of the matmul).  Only those rows need the region MIN (for them
  the region columns are positives); every row needs the region MAX.

  Device (per core, 1/8 of the columns = 8192, padded w/ duplicates):
    fp8e4 DoubleRow matmuls: psum = (16*q)·(16*z) = 256*p, 32 units of
    [128 rows, 2 banks=1024 cols].  Per unit one of:
      M (m-block 0): ACT exp(T/256*psum)+accum -> LSE max est;
                     DVE tensor_reduce min on psum -> exact min
      A: ACT exp+accum only (LSE max estimate)
      P: DVE tensor_reduce max on psum (exact max)
  Host combines: max-estimate = max over units/cores of log(sum)/T or
  raw TR max (/256); min likewise from M units.  Measured loss error is
  ~1e-5 relative (gate is 2e-2); LSE bias ~0.12 in p units largely
  cancels through the sqrt/softplus chain.
  Measured: ~47-48.3us HW exec (traced) vs 93.7us for the previous
  kernel under identical measurement (79.9us untraced harness number).
"""

import sys
import types
import numpy as np
import ml_dtypes

N, DIM, K, B = 512, 512, 65536, 512
NCORES = 8
KZ = K - B              # zero-label columns
CPC = K // NCORES       # padded columns per core (8192)
NT = CPC // 512         # 512-wide column tiles per core
BIG = 9999999.0

SQ = 16.0               # fp8 scale for feat_q
SZ = 16.0               # fp8 scale for queue columns
T = 8.0                 # LSE temperature (p units)
ACT_SCALE = T / (SQ * SZ)

FP8 = ml_dtypes.float8_e4m3

# types for the 6 non-m0 units per block, unit order
# [(1,0),(2,0),(3,0),(1,1),(2,1),(3,1)]
BLOCK_TYPES = {
    0: list("APPAPP"),
    1: list("APAPAP"),
    2: list("APAPAP"),
    3: list("APAPAP"),
}

LAST_RESULTS = None     # BassKernelResults of the most recent device run
_NC_CACHE = {}


def _install_axon_hooks_shim():
    """antenv.axon_hooks is absent on this image; bass_utils imports it when
    NTFF tracing is requested.  Provide the get/set module and register the
    ctypes-based NTFF hook so trace=True / BASS_TRACE=1 works."""
    try:
        import antenv  # noqa: F401
    except ImportError:
        return
    if "antenv.axon_hooks" in sys.modules:
        return
    mod = types.ModuleType("antenv.axon_hooks")
    mod._hook = None

    def set_axon_ntff_profile_hook(h):
        mod._hook = h

    def get_axon_ntff_profile_hook():
        return mod._hook

    mod.set_axon_ntff_profile_hook = set_axon_ntff_profile_hook
    mod.get_axon_ntff_profile_hook = get_axon_ntff_profile_hook
    sys.modules["antenv.axon_hooks"] = mod
    sys.modules["antenv"].axon_hooks = mod
    try:
        from trn_agent_boot.trn_boot import _ntff_profile_via_ctypes

        mod._hook = _ntff_profile_via_ctypes("/opt/axon/libaxon_pjrt.so")
    except Exception:
        pass


def _unit_list(nb):
    """units in production order: (m, h, type)"""
    out = []
    for h in range(2):
        for m in range(4):
            if m == 0:
                ty = "M"
            else:
                ty = BLOCK_TYPES[nb][(m - 1) + 3 * h]
            out.append((m, h, ty))
    return out


def _build_nc():
    import concourse.bacc as bacc
    import concourse.mybir as mybir
    from concourse.tile import TileContext

    fp8 = mybir.dt.float8e4
    bf16 = mybir.dt.bfloat16
    f32 = mybir.dt.float32

    nc = bacc.Bacc("TRN2", debug=False, target_bir_lowering=False)
    qt_d = nc.dram_tensor("qt", [128, 4 * 512], fp8, kind="ExternalInput")
    st_d = nc.dram_tensor("st", [128, NT * 4 * 512], fp8, kind="ExternalInput")
    out_d = nc.dram_tensor("out", [128, 96], f32, kind="ExternalOutput")

    qt_v = qt_d.ap().rearrange("p (k m) -> p k m", k=4)
    st_v = st_d.ap().rearrange("p (j t k c) -> p j t k c", j=4, t=4, k=4)

    with TileContext(nc) as tc:
        with (
            tc.tile_pool(name="qpool", bufs=1) as qpool,
            tc.tile_pool(name="spool", bufs=8) as spool,
            tc.tile_pool(name="bpool", bufs=2) as bpool,
            tc.tile_pool(name="opool", bufs=1) as opool,
            tc.tile_pool(name="pspool", bufs=4, space="PSUM") as pspool,
        ):
            qt = qpool.tile([128, 4, 512], fp8, name="qt")
            stq = [
                spool.tile([128, 4, 4, 512], fp8, name="stq", tag="stq", bufs=4)
                for _ in range(4)
            ]
            nc.sync.dma_start(out=qt, in_=qt_v)
            # first quad split in halves so block 0 can start early
            nc.sync.dma_start(out=stq[0][:, 0:2, :, :], in_=st_v[:, 0, 0:2, :, :])
            nc.sync.dma_start(out=stq[0][:, 2:4, :, :], in_=st_v[:, 0, 2:4, :, :])
            for j in range(1, 4):
                nc.sync.dma_start(out=stq[j], in_=st_v[:, j, :, :, :])

            # HAM warmup: wide cold matmuls filling the DMA-wait window
            warm = qpool.tile([128, 16], bf16, name="warm")
            warm2 = qpool.tile([128, 512], bf16, name="warm2")
            nc.vector.memset(warm, 0.0)
            nc.vector.memset(warm2, 0.0)
            wps = pspool.tile([128, 2, 512], f32, name="wps", tag="ps")
            for _ in range(9):
                nc.tensor.matmul(wps[0:16, 0, :], warm, warm2)
            # preload the exp activation table during the DMA wait
            wexpo = qpool.tile([128, 8], bf16, name="wexpo")
            nc.scalar.activation(wexpo, warm[:, 0:8],
                                 mybir.ActivationFunctionType.Exp)

            outt = opool.tile([128, 96], f32, name="outt")
            nc.gpsimd.memset(outt, 0.0)
            osum = outt[:, 0:32]
            ored = outt[:, 32:96]

            for nb in range(4):
                for (m, h, ty) in _unit_list(nb):
                    u = nb * 8 + m * 2 + h
                    ps = pspool.tile([128, 2, 512], f32, name="ps", tag="ps")
                    for kp in range(2):
                        for nn in range(2):
                            nc.tensor.matmul(
                                ps[:, nn, :],
                                qt[:, 2 * kp : 2 * kp + 2, m * 128 : (m + 1) * 128],
                                stq[nb][:, 2 * h + nn, 2 * kp : 2 * kp + 2, :],
                                start=(kp == 0),
                                stop=(kp == 1),
                                perf_mode=mybir.MatmulPerfMode.DoubleRow,
                            )
                    if ty in ("M", "A"):
                        bt = bpool.tile([128, 2, 512], bf16, name="bt", tag="bt")
                        nc.scalar.activation(
                            bt, ps, mybir.ActivationFunctionType.Exp,
                            scale=ACT_SCALE, accum_out=osum[:, u : u + 1],
                        )
                    if ty == "M":
                        # min of exp tile (SBUF) = exp(min); psum released by ACT
                        nc.vector.tensor_reduce(
                            ored[:, 2 * u : 2 * u + 2], bt,
                            axis=mybir.AxisListType.X, op=mybir.AluOpType.min,
                        )
                    elif ty == "P":
                        nc.vector.tensor_reduce(
                            ored[:, 2 * u : 2 * u + 2], ps,
                            axis=mybir.AxisListType.X, op=mybir.AluOpType.max,
                        )

            nc.scalar.dma_start(out=out_d.ap(), in_=outt)

    nc.compile()
    return nc


def _get_nc():
    if "nc" not in _NC_CACHE:
        _install_axon_hooks_shim()
        _NC_CACHE["nc"] = _build_nc()
    return _NC_CACHE["nc"]


def _loss(dist_ap, dist_an):
    diff = dist_an - dist_ap
    loss_soft = np.mean(np.logaddexp(0.0, -diff))
    if np.isinf(loss_soft):
        return np.float32(np.mean(np.maximum(dist_ap - dist_an + 0.3, 0.0)))
    return np.float32(loss_soft)


def _host_reference(feat_q, feat_k, targets, queue, queue_label):
    """Exact numpy fallback (float64) — used if structural assumptions or
    numeric sanity checks fail."""
    fq = feat_q.astype(np.float64)
    fk = feat_k.astype(np.float64)
    t = targets.astype(np.int64)
    q = queue.astype(np.float64).copy()
    ql = queue_label.astype(np.int64).copy()
    q[:, : fk.shape[0]] = fk.T
    ql[: fk.shape[0]] = t
    xx = (fq * fq).sum(1)[:, None]
    yy = (q * q).sum(0)[None, :]
    sq = xx + yy - 2.0 * (fq @ q)
    dist = np.sqrt(np.clip(sq, 1e-12, None))
    is_pos = t[:, None] == ql[None, :]
    dist_ap = np.max(dist - BIG * (~is_pos), axis=1)
    dist_an = np.min(dist + BIG * is_pos, axis=1)
    return _loss(dist_ap, dist_an)


def kernel(feat_q, feat_k, targets, queue, queue_label):
    feat_q = np.asarray(feat_q, dtype=np.float32)
    feat_k = np.asarray(feat_k, dtype=np.float32)
    targets = np.asarray(targets)
    queue = np.asarray(queue, dtype=np.float32)
    queue_label = np.asarray(queue_label)

    t = targets.astype(np.int64)
    Z = queue[:, B:]  # zero-label region, untouched by the enqueue

    # Guards for the structural assumptions this split relies on.
    ok = not np.any(queue_label != 0)
    if ok:
        sample = np.linspace(0, KZ - 1, 512, dtype=np.int64)
        yy_s = np.einsum("ij,ij->j", Z[:, sample], Z[:, sample], dtype=np.float64)
        ok = bool(np.max(np.abs(yy_s - 1.0)) < 1e-3)
    # Row permutation: target==0 rows first (they are the only ones needing
    # the region min, computed on m-block 0 of the device matmul).
    t0_rows = np.where(t == 0)[0]
    rest = np.where(t != 0)[0]
    if ok:
        ok = len(t0_rows) <= 128
    if not ok:
        return _host_reference(feat_q, feat_k, targets, queue, queue_label)

    perm = np.concatenate([t0_rows, rest])
    n0 = len(t0_rows)
    fq_p = feat_q[perm]
    t_p = t[perm]

    # ---- device inputs ----
    q8 = (SQ * fq_p.T).astype(FP8)          # [512 d, 512 i]
    qt_host = np.ascontiguousarray(
        q8.reshape(4, 128, 512).transpose(1, 0, 2).reshape(128, 4 * 512)
    )
    Z8 = (SZ * Z).astype(FP8)               # [512, 65024]
    in_maps = []
    for c in range(NCORES):
        lo = c * CPC
        hi = min((c + 1) * CPC, KZ)
        sl = np.empty((DIM, CPC), dtype=FP8)
        sl[:, : hi - lo] = Z8[:, lo:hi]
        if hi - lo < CPC:  # pad the tail core with duplicate columns
            sl[:, hi - lo :] = Z8[:, : CPC - (hi - lo)]
        st_host = np.ascontiguousarray(
            sl.reshape(4, 128, NT, 512).transpose(1, 2, 0, 3)
            .reshape(128, NT * 4 * 512)
        )
        in_maps.append({"qt": qt_host, "st": st_host})

    from concourse import bass_utils

    nc = _get_nc()
    res = bass_utils.run_bass_kernel_spmd(nc, in_maps, core_ids=list(range(NCORES)))
    global LAST_RESULTS
    LAST_RESULTS = res

    # ---- decode device outputs: raw units are 256*p ----
    pmax_raw = np.full((128, 4), -np.inf)
    pmin0_raw = np.full(128, np.inf)
    sums_ok = True
    for c in range(NCORES):
        outt = np.asarray(res.results[c]["out"], dtype=np.float64)
        osum = outt[:, 0:32]
        ored = outt[:, 32:96]
        for nb in range(4):
            for (m, h, ty) in _unit_list(nb):
                u = nb * 8 + m * 2 + h
                if ty in ("M", "A"):
                    s = osum[:, u]
                    if not np.all(np.isfinite(s)) or np.any(s <= 0.0):
                        sums_ok = False
                    else:
                        pmax_raw[:, m] = np.maximum(
                            pmax_raw[:, m], np.log(s) / T * (SQ * SZ)
                        )
                else:
                    pmax_raw[:, m] = np.maximum(
                        pmax_raw[:, m], ored[:, 2 * u : 2 * u + 2].max(axis=1)
                    )
                if ty == "M":
                    me = ored[:, 2 * u : 2 * u + 2].min(axis=1)
                    if np.any(me <= 0.0) or not np.all(np.isfinite(me)):
                        sums_ok = False
                    else:
                        pmin0_raw = np.minimum(
                            pmin0_raw, np.log(me) / T * (SQ * SZ)
                        )
    if not sums_ok or not np.all(np.isfinite(pmax_raw)):
        return _host_reference(feat_q, feat_k, targets, queue, queue_label)

    pmax = pmax_raw.T.reshape(N) / (SQ * SZ)     # row i_p = m*128 + p
    pmin0 = pmin0_raw / (SQ * SZ)                # rows 0..127 (m-block 0)

    # ---- host part: special 512-column block, exact in float64 ----
    fq = fq_p.astype(np.float64)
    fk = feat_k.astype(np.float64)
    xx = (fq * fq).sum(1)
    kk_ = (fk * fk).sum(1)
    G = fq @ fk.T
    sqB = xx[:, None] + kk_[None, :] - 2.0 * G
    distB = np.sqrt(np.clip(sqB, 1e-12, None))
    maskB = t_p[:, None] == t[None, :]           # special-block labels = targets
    apB = np.max(distB - BIG * (~maskB), axis=1)
    anB = np.min(distB + BIG * maskB, axis=1)

    # region contributions (columns are all label 0, unit norm)
    # rows with t_p != 0: region is negative -> min distance from pmax
    d_zmin = np.sqrt(np.clip(xx + 1.0 - 2.0 * pmax, 1e-12, None))
    an_z = d_zmin + BIG * (t_p == 0)
    # rows with t_p == 0 (first n0 rows): region is positive -> max distance
    ap_z = np.full(N, -BIG)
    if n0 > 0:
        d_zmax0 = np.sqrt(np.clip(xx[:n0] + 1.0 - 2.0 * pmin0[:n0], 1e-12, None))
        ap_z[:n0] = d_zmax0

    dist_ap = np.maximum(apB, ap_z)
    dist_an = np.minimum(anB, an_z)
    return _loss(dist_ap, dist_an)


# revision 20
# speedup vs baseline: 1.0814x; 1.0216x over previous
"""MoCo hard-example-mining loss (topk_masking) on 8 Trainium2 NeuronCores.

Strategy (sharding_hint: shard queue along K):
  After the enqueue step the effective queue is [feat_k.T | Z] where
  Z = queue[:, 512:] are L2-normalized columns with label 0.  The loss only
  needs, per feat_q row i, the max and min of p_ij = <feat_q_i, z_j> over
  the zero-label region (64.5K columns), plus exact handling of the 512
  special columns (done on the host in float64).

  Rows are PERMUTED on the host so that all rows with target==0 come first
  (m-block 0 of the matmul).  Only those rows need the region MIN (for them
  the region columns are positives); every row needs the region MAX.

  Device (per core, 1/8 of the columns = 8192, padded w/ duplicates):
    fp8e4 DoubleRow matmuls: psum = (16*q)·(16*z) = 256*p, 32 units of
    [128 rows, 2 banks=1024 cols].  Per unit one of:
      M (m-block 0): ACT exp(T/256*psum)+accum -> LSE max est;
                     DVE tensor_reduce min on psum -> exact min
      A: ACT exp+accum only (LSE max estimate)
      P: DVE tensor_reduce max on psum (exact max)
  Host combines: max-estimate = max over units/cores of log(sum)/T or
  raw TR max (/256); min likewise from M units.  Measured loss error is
  ~1e-5 relative (gate is 2e-2); LSE bias ~0.12 in p units largely
  cancels through the sqrt/softplus chain.
  Measured: ~47-48.3us HW exec (traced) vs 93.7us for the previous
  kernel under identical measurement (79.9us untraced harness number).
"""

import sys
import types
import numpy as np
import ml_dtypes

N, DIM, K, B = 512, 512, 65536, 512
NCORES = 8
KZ = K - B              # zero-label columns
CPC = K // NCORES       # padded columns per core (8192)
NT = CPC // 512         # 512-wide column tiles per core
BIG = 9999999.0

SQ = 16.0               # fp8 scale for feat_q
SZ = 16.0               # fp8 scale for queue columns
T = 8.0                 # LSE temperature (p units)
ACT_SCALE = T / (SQ * SZ)

FP8 = ml_dtypes.float8_e4m3

# types for the 6 non-m0 units per block, unit order
# [(1,0),(2,0),(3,0),(1,1),(2,1),(3,1)]
BLOCK_TYPES = {
    0: list("APAPAP"),
    1: list("APAPAP"),
    2: list("APPAPP"),
    3: list("APAPAP"),
}

LAST_RESULTS = None     # BassKernelResults of the most recent device run
_NC_CACHE = {}


def _install_axon_hooks_shim():
    """antenv.axon_hooks is absent on this image; bass_utils imports it when
    NTFF tracing is requested.  Provide the get/set module and register the
    ctypes-based NTFF hook so trace=True / BASS_TRACE=1 works."""
    try:
        import antenv  # noqa: F401
    except ImportError:
        return
    if "antenv.axon_hooks" in sys.modules:
        return
    mod = types.ModuleType("antenv.axon_hooks")
    mod._hook = None

    def set_axon_ntff_profile_hook(h):
        mod._hook = h

    def get_axon_ntff_profile_hook():
        return mod._hook

    mod.set_axon_ntff_profile_hook = set_axon_ntff_profile_hook
    mod.get_axon_ntff_profile_hook = get_axon_ntff_profile_hook
    sys.modules["antenv.axon_hooks"] = mod
    sys.modules["antenv"].axon_hooks = mod
    try:
        from trn_agent_boot.trn_boot import _ntff_profile_via_ctypes

        mod._hook = _ntff_profile_via_ctypes("/opt/axon/libaxon_pjrt.so")
    except Exception:
        pass


def _unit_list(nb):
    """units in production order: (m, h, type)"""
    out = []
    for h in range(2):
        for m in range(4):
            if m == 0:
                ty = "M"
            else:
                ty = BLOCK_TYPES[nb][(m - 1) + 3 * h]
            out.append((m, h, ty))
    return out


def _build_nc():
    import concourse.bacc as bacc
    import concourse.mybir as mybir
    from concourse.tile import TileContext

    fp8 = mybir.dt.float8e4
    bf16 = mybir.dt.bfloat16
    f32 = mybir.dt.float32

    nc = bacc.Bacc("TRN2", debug=False, target_bir_lowering=False)
    qt_d = nc.dram_tensor("qt", [128, 4 * 512], fp8, kind="ExternalInput")
    st_d = nc.dram_tensor("st", [128, NT * 4 * 512], fp8, kind="ExternalInput")
    out_d = nc.dram_tensor("out", [128, 96], f32, kind="ExternalOutput")

    qt_v = qt_d.ap().rearrange("p (k m) -> p k m", k=4)
    st_v = st_d.ap().rearrange("p (j t k c) -> p j t k c", j=4, t=4, k=4)

    with TileContext(nc) as tc:
        with (
            tc.tile_pool(name="qpool", bufs=1) as qpool,
            tc.tile_pool(name="spool", bufs=8) as spool,
            tc.tile_pool(name="bpool", bufs=2) as bpool,
            tc.tile_pool(name="opool", bufs=1) as opool,
            tc.tile_pool(name="pspool", bufs=4, space="PSUM") as pspool,
        ):
            qt = qpool.tile([128, 4, 512], fp8, name="qt")
            stq = [
                spool.tile([128, 4, 4, 512], fp8, name="stq", tag="stq", bufs=4)
                for _ in range(4)
            ]
            nc.sync.dma_start(out=qt, in_=qt_v)
            # first quad split in halves so block 0 can start early
            nc.sync.dma_start(out=stq[0][:, 0:2, :, :], in_=st_v[:, 0, 0:2, :, :])
            nc.sync.dma_start(out=stq[0][:, 2:4, :, :], in_=st_v[:, 0, 2:4, :, :])
            for j in range(1, 4):
                nc.sync.dma_start(out=stq[j], in_=st_v[:, j, :, :, :])

            # HAM warmup: wide cold matmuls filling the DMA-wait window
            warm = qpool.tile([128, 16], bf16, name="warm")
            warm2 = qpool.tile([128, 512], bf16, name="warm2")
            nc.vector.memset(warm, 0.0)
            nc.vector.memset(warm2, 0.0)
            wps = pspool.tile([128, 2, 512], f32, name="wps", tag="ps")
            for _ in range(9):
                nc.tensor.matmul(wps[0:16, 0, :], warm, warm2)
            # preload the exp activation table during the DMA wait
            wexpo = qpool.tile([128, 8], bf16, name="wexpo")
            nc.scalar.activation(wexpo, warm[:, 0:8],
                                 mybir.ActivationFunctionType.Exp)

            outt = opool.tile([128, 96], f32, name="outt")
            nc.gpsimd.memset(outt, 0.0)
            osum = outt[:, 0:32]
            ored = outt[:, 32:96]

            for nb in range(4):
                for (m, h, ty) in _unit_list(nb):
                    u = nb * 8 + m * 2 + h
                    ps = pspool.tile([128, 2, 512], f32, name="ps", tag="ps")
                    for kp in range(2):
                        for nn in range(2):
                            nc.tensor.matmul(
                                ps[:, nn, :],
                                qt[:, 2 * kp : 2 * kp + 2, m * 128 : (m + 1) * 128],
                                stq[nb][:, 2 * h + nn, 2 * kp : 2 * kp + 2, :],
                                start=(kp == 0),
                                stop=(kp == 1),
                                perf_mode=mybir.MatmulPerfMode.DoubleRow,
                            )
                    if ty in ("M", "A"):
                        bt = bpool.tile([128, 2, 512], bf16, name="bt", tag="bt")
                        nc.scalar.activation(
                            bt, ps, mybir.ActivationFunctionType.Exp,
                            scale=ACT_SCALE, accum_out=osum[:, u : u + 1],
                        )
                    if ty == "M":
                        # min of exp tile (SBUF) = exp(min); psum released by ACT
                        nc.vector.tensor_reduce(
                            ored[:, 2 * u : 2 * u + 2], bt,
                            axis=mybir.AxisListType.X, op=mybir.AluOpType.min,
                        )
                    elif ty == "P":
                        nc.vector.tensor_reduce(
                            ored[:, 2 * u : 2 * u + 2], ps,
                            axis=mybir.AxisListType.X, op=mybir.AluOpType.max,
                        )

            nc.scalar.dma_start(out=out_d.ap(), in_=outt)

    nc.compile()
    return nc


def _get_nc():
    if "nc" not in _NC_CACHE:
        _install_axon_hooks_shim()
        _NC_CACHE["nc"] = _build_nc()
    return _NC_CACHE["nc"]


def _loss(dist_ap, dist_an):
    diff = dist_an - dist_ap
    loss_soft = np.mean(np.logaddexp(0.0, -diff))
    if np.isinf(loss_soft):
        return np.float32(np.mean(np.maximum(dist_ap - dist_an + 0.3, 0.0)))
    return np.float32(loss_soft)


def _host_reference(feat_q, feat_k, targets, queue, queue_label):
    """Exact numpy fallback (float64) — used if structural assumptions or
    numeric sanity checks fail."""
    fq = feat_q.astype(np.float64)
    fk = feat_k.astype(np.float64)
    t = targets.astype(np.int64)
    q = queue.astype(np.float64).copy()
    ql = queue_label.astype(np.int64).copy()
    q[:, : fk.shape[0]] = fk.T
    ql[: fk.shape[0]] = t
    xx = (fq * fq).sum(1)[:, None]
    yy = (q * q).sum(0)[None, :]
    sq = xx + yy - 2.0 * (fq @ q)
    dist = np.sqrt(np.clip(sq, 1e-12, None))
    is_pos = t[:, None] == ql[None, :]
    dist_ap = np.max(dist - BIG * (~is_pos), axis=1)
    dist_an = np.min(dist + BIG * is_pos, axis=1)
    return _loss(dist_ap, dist_an)


def kernel(feat_q, feat_k, targets, queue, queue_label):
    feat_q = np.asarray(feat_q, dtype=np.float32)
    feat_k = np.asarray(feat_k, dtype=np.float32)
    targets = np.asarray(targets)
    queue = np.asarray(queue, dtype=np.float32)
    queue_label = np.asarray(queue_label)

    t = targets.astype(np.int64)
    Z = queue[:, B:]  # zero-label region, untouched by the enqueue

    # Guards for the structural assumptions this split relies on.
    ok = not np.any(queue_label != 0)
    if ok:
        sample = np.linspace(0, KZ - 1, 512, dtype=np.int64)
        yy_s = np.einsum("ij,ij->j", Z[:, sample], Z[:, sample], dtype=np.float64)
        ok = bool(np.max(np.abs(yy_s - 1.0)) < 1e-3)
    # Row permutation: target==0 rows first (they are the only ones needing
    # the region min, computed on m-block 0 of the device matmul).
    t0_rows = np.where(t == 0)[0]
    rest = np.where(t != 0)[0]
    if ok:
        ok = len(t0_rows) <= 128
    if not ok:
        return _host_reference(feat_q, feat_k, targets, queue, queue_label)

    perm = np.concatenate([t0_rows, rest])
    n0 = len(t0_rows)
    fq_p = feat_q[perm]
    t_p = t[perm]

    # ---- device inputs ----
    q8 = (SQ * fq_p.T).astype(FP8)          # [512 d, 512 i]
    qt_host = np.ascontiguousarray(
        q8.reshape(4, 128, 512).transpose(1, 0, 2).reshape(128, 4 * 512)
    )
    Z8 = (SZ * Z).astype(FP8)               # [512, 65024]
    in_maps = []
    for c in range(NCORES):
        lo = c * CPC
        hi = min((c + 1) * CPC, KZ)
        sl = np.empty((DIM, CPC), dtype=FP8)
        sl[:, : hi - lo] = Z8[:, lo:hi]
        if hi - lo < CPC:  # pad the tail core with duplicate columns
            sl[:, hi - lo :] = Z8[:, : CPC - (hi - lo)]
        st_host = np.ascontiguousarray(
            sl.reshape(4, 128, NT, 512).transpose(1, 2, 0, 3)
            .reshape(128, NT * 4 * 512)
        )
        in_maps.append({"qt": qt_host, "st": st_host})

    from concourse import bass_utils

    nc = _get_nc()
    res = bass_utils.run_bass_kernel_spmd(nc, in_maps, core_ids=list(range(NCORES)))
    global LAST_RESULTS
    LAST_RESULTS = res

    # ---- decode device outputs: raw units are 256*p ----
    pmax_raw = np.full((128, 4), -np.inf)
    pmin0_raw = np.full(128, np.inf)
    sums_ok = True
    for c in range(NCORES):
        outt = np.asarray(res.results[c]["out"], dtype=np.float64)
        osum = outt[:, 0:32]
        ored = outt[:, 32:96]
        for nb in range(4):
            for (m, h, ty) in _unit_list(nb):
                u = nb * 8 + m * 2 + h
                if ty in ("M", "A"):
                    s = osum[:, u]
                    if not np.all(np.isfinite(s)) or np.any(s <= 0.0):
                        sums_ok = False
                    else:
                        pmax_raw[:, m] = np.maximum(
                            pmax_raw[:, m], np.log(s) / T * (SQ * SZ)
                        )
                else:
                    pmax_raw[:, m] = np.maximum(
                        pmax_raw[:, m], ored[:, 2 * u : 2 * u + 2].max(axis=1)
                    )
                if ty == "M":
                    me = ored[:, 2 * u : 2 * u + 2].min(axis=1)
                    if np.any(me <= 0.0) or not np.all(np.isfinite(me)):
                        sums_ok = False
                    else:
                        pmin0_raw = np.minimum(
                            pmin0_raw, np.log(me) / T * (SQ * SZ)
                        )
    if not sums_ok or not np.all(np.isfinite(pmax_raw)):
        return _host_reference(feat_q, feat_k, targets, queue, queue_label)

    pmax = pmax_raw.T.reshape(N) / (SQ * SZ)     # row i_p = m*128 + p
    pmin0 = pmin0_raw / (SQ * SZ)                # rows 0..127 (m-block 0)

    # ---- host part: special 512-column block, exact in float64 ----
    fq = fq_p.astype(np.float64)
    fk = feat_k.astype(np.float64)
    xx = (fq * fq).sum(1)
    kk_ = (fk * fk).sum(1)
    G = fq @ fk.T
    sqB = xx[:, None] + kk_[None, :] - 2.0 * G
    distB = np.sqrt(np.clip(sqB, 1e-12, None))
    maskB = t_p[:, None] == t[None, :]           # special-block labels = targets
    apB = np.max(distB - BIG * (~maskB), axis=1)
    anB = np.min(distB + BIG * maskB, axis=1)

    # region contributions (columns are all label 0, unit norm)
    # rows with t_p != 0: region is negative -> min distance from pmax
    d_zmin = np.sqrt(np.clip(xx + 1.0 - 2.0 * pmax, 1e-12, None))
    an_z = d_zmin + BIG * (t_p == 0)
    # rows with t_p == 0 (first n0 rows): region is positive -> max distance
    ap_z = np.full(N, -BIG)
    if n0 > 0:
        d_zmax0 = np.sqrt(np.clip(xx[:n0] + 1.0 - 2.0 * pmin0[:n0], 1e-12, None))
        ap_z[:n0] = d_zmax0

    dist_ap = np.maximum(apB, ap_z)
    dist_an = np.minimum(anB, an_z)
    return _loss(dist_ap, dist_an)
